# revision 13
# baseline (speedup 1.0000x reference)
"""MixLinear GEMM kernel for Trainium2 (8 NeuronCores, column-parallel).

Computes, for full inputs:
    inputs = x.reshape(-1, 4096)
    act_outliers = inputs[:, ind]
    inputs_z = inputs with ind-columns zeroed
    x_scale = clamp(rowmax(|inputs_z|)/127, 1e-8)
    q_x = round(inputs_z / x_scale)                  (|q_x| <= 127 by construction)
    y = (q_x @ q_weight.T) * x_scale * scale_col + act_outliers @ weight_cache.T + bias

Sharding: q_weight/scale_col/weight_cache/bias are sharded along out_features
across the 8 cores (column parallel); x is replicated. Each core produces its
(512, 1376) output shard; the host concatenates.

Kernel design (v2):
- The outlier GEMM is folded into the main GEMM on the host: the transposed
  weight matrix wT[k, o] gets its ind-rows REPLACED by weight_cache[:, j]/sc
  (summed over duplicate indices). The device quantizes the UNMASKED x, so
  q'[k in ind] = round(x_ind/xs) and the single GEMM produces
  y_int + outliers/(xs*sc) in one pass. absmax is computed over MASKED x via
  a fused tensor_tensor_reduce with a 0/1 mask.
- Weights are pre-transposed/packed on the host to [128, KT, OSH] f16 and stay
  resident in SBUF; no on-device weight transposes.
- Activations are quantized in natural layout (ACT engine applies
  x*recip + 1536; the fp16 write rounds to integer), transposed 128x128 at a
  time on the PE against an identity, and fixed up (-1536) during the
  PSUM->SBUF copy on DVE.
- Main GEMM: for each m-tile, 3 output chunks (512/512/352 wide), 32 matmuls
  of 128-contraction each accumulating in one PSUM bank. Epilogue on DVE:
  (psum * xs) * sc + bias, then DMA out.
"""

import sys

import numpy as np

sys.path.insert(0, "/opt/trn_rl_repo")

import concourse.bass as bass  # noqa: E402
import concourse.mybir as mybir  # noqa: E402
import concourse.tile as tile  # noqa: E402
from concourse import bacc  # noqa: E402

N_CORES = 8
M = 512  # 8*64 rows
K = 4096  # in_features
OUT = 11008  # out_features
OSH = OUT // N_CORES  # 1376 per-core shard
FP = 256  # outlier columns
KT = K // 128  # 32 k-tiles
MT = M // 128  # 4 m-tiles
MAGIC = 1536.0  # fp16 spacing is 1.0 in [1024, 2048): forces round-to-int
XH = 2048  # x processed in half-rows (SBUF economy)
CHUNKS = [(0, 512), (512, 512), (1024, 352)]  # (o0, cw) output chunks
WG = 8  # k-tile groups per weight-load DMA

f32 = mybir.dt.float32
f16 = mybir.dt.float16
bf16 = mybir.dt.bfloat16
Alu = mybir.AluOpType
Act = mybir.ActivationFunctionType


def build_program(nrep=1, debug_dump=False):
    """Build the kernel program. nrep>1 emits the whole body nrep times
    (same inputs, same outputs) - used only to measure steady-state HW time
    as (t(nrep) - t(1)) / (nrep - 1)."""
    nc = bacc.Bacc(
        "TRN2", target_bir_lowering=False, debug=False, num_devices=N_CORES
    )

    x_d = nc.dram_tensor("x_in", [M, K], f32, kind="ExternalInput").ap()
    w_d = nc.dram_tensor("w_in", [128, KT * OSH], f16, kind="ExternalInput").ap()
    mask_d = nc.dram_tensor("mask_in", [1, K], bf16, kind="ExternalInput").ap()
    sc_d = nc.dram_tensor("sc_in", [1, OSH], f32, kind="ExternalInput").ap()
    bias_d = nc.dram_tensor("bias_in", [1, OSH], f32, kind="ExternalInput").ap()
    y_d = nc.dram_tensor("y_out", [M, OSH], f32, kind="ExternalOutput").ap()
    dbg = {}
    if debug_dump:
        for nm, shape, dt in [
            ("dbg_scales", [128, 3 * MT], f32),
            ("dbg_q0", [128, KT * 128], f16),
            ("dbg_q3", [128, KT * 128], f16),
            ("dbg_w0", [128, OSH], f16),
            ("dbg_w31", [128, OSH], f16),
        ]:
            dbg[nm] = nc.dram_tensor(nm, shape, dt, kind="ExternalOutput").ap()

    with tile.TileContext(nc) as tc:
        with (
            tc.tile_pool(name="persist", bufs=1) as persist,
            tc.tile_pool(name="xpool", bufs=4) as xpool,
            tc.tile_pool(name="qnpool", bufs=4) as qnpool,
            tc.tile_pool(name="ypool", bufs=3) as ypool,
            tc.tile_pool(name="psmain", bufs=4, space="PSUM") as psmain,
        ):
            # ---------- persistent tiles ----------
            w_sb = persist.tile([128, KT, OSH], f16)  # resident weights^T
            mask_bc = persist.tile([128, K], bf16)
            sc_bc = persist.tile([128, OSH], f32)
            bias_bc = persist.tile([128, OSH], f32)
            q_tiles = [
                persist.tile([128, KT, 128], f16, tag=f"qT{mt}", name=f"qT{mt}")
                for mt in range(MT)
            ]
            am_all = persist.tile([128, MT], f32)
            am_h = persist.tile([128, 2 * MT], f32)
            xs_all = persist.tile([128, MT], f32)
            recip_all = persist.tile([128, MT], f32)
            xz_scr = persist.tile([128, XH], f32)  # ttr dst (never read)

            # ---------- setup ----------
            # resident weights: 4 big HWDGE loads, f16, no transpose needed
            for g in range(KT // WG):
                nc.sync.dma_start(
                    out=w_sb[:, g * WG : (g + 1) * WG, :],
                    in_=w_d[:, g * WG * OSH : (g + 1) * WG * OSH],
                )
            # broadcasts across partitions: DRAM AP with partition-step 0
            nc.gpsimd.dma_start(
                out=mask_bc,
                in_=bass.AP(mask_d.tensor, mask_d.offset, [[0, 128], [1, K]]),
            )
            nc.gpsimd.dma_start(
                out=sc_bc,
                in_=bass.AP(sc_d.tensor, sc_d.offset, [[0, 128], [1, OSH]]),
            )
            nc.gpsimd.dma_start(
                out=bias_bc,
                in_=bass.AP(bias_d.tensor, bias_d.offset, [[0, 128], [1, OSH]]),
            )
            def phase1(rep, mt):
                """x load -> masked absmax -> quantize -> XBAR transpose."""
                ms = slice(mt * 128, (mt + 1) * 128)
                x_hs = []
                for h in range(2):
                    x_h = xpool.tile(
                        [128, XH], f32, tag="x", name=f"x_{rep}_{mt}_{h}"
                    )
                    nc.scalar.dma_start(
                        out=x_h, in_=x_d[ms, h * XH : (h + 1) * XH]
                    )
                    x_hs.append(x_h)
                    nc.gpsimd.tensor_tensor(
                        out=xz_scr,
                        in0=x_h,
                        in1=mask_bc[:, h * XH : (h + 1) * XH],
                        op=Alu.mult,
                    )
                    nc.vector.tensor_reduce(
                        out=am_h[:, 2 * mt + h : 2 * mt + h + 1],
                        in_=xz_scr,
                        axis=mybir.AxisListType.X,
                        op=Alu.max,
                        apply_absolute_value=True,
                    )
                nc.vector.tensor_reduce(
                    out=am_all[:, mt : mt + 1],
                    in_=am_h[:, 2 * mt : 2 * mt + 2],
                    axis=mybir.AxisListType.X,
                    op=Alu.max,
                    apply_absolute_value=False,
                )
                # xs = max(absmax/127, 1e-8); recip = 1/xs
                nc.vector.tensor_scalar(
                    xs_all[:, mt : mt + 1],
                    am_all[:, mt : mt + 1],
                    1.0 / 127.0,
                    1e-8,
                    Alu.mult,
                    Alu.max,
                )
                nc.vector.reciprocal(
                    out=recip_all[:, mt : mt + 1], in_=xs_all[:, mt : mt + 1]
                )
                q_t = q_tiles[mt]
                for h in range(2):
                    # q_off = x*recip + 1536 -> fp16 write rounds to int (RNE)
                    qn = qnpool.tile(
                        [128, XH], f16, tag="qn", name=f"qn_{rep}_{mt}_{h}"
                    )
                    nc.scalar.activation(
                        out=qn,
                        in_=x_hs[h],
                        func=Act.Copy,
                        bias=MAGIC,
                        scale=recip_all[:, mt : mt + 1],
                    )
                    # XBAR transpose into the k-partition layout.
                    # NOTE: must be issued from the SP sequencer -
                    # ACT-issued xbar transposes corrupt data on HW.
                    nc.sync.dma_start(
                        out=q_t[:, h * (XH // 128) : (h + 1) * (XH // 128), :],
                        in_=qn,
                        transpose=True,
                    )
                # undo the rounding bias in place: q = q_off - 1536
                nc.vector.tensor_scalar(
                    q_t[:, :, :], q_t[:, :, :], MAGIC, None, Alu.subtract
                )

            def phase2(rep, mt):
                """main GEMM over output chunks + epilogue."""
                ms = slice(mt * 128, (mt + 1) * 128)
                q_t = q_tiles[mt]
                for o0, cw in CHUNKS:
                    ps = psmain.tile([128, 512], f32, tag="ps")
                    for kk in range(KT):
                        nc.tensor.matmul(
                            ps[:, :cw],
                            lhsT=q_t[:, kk, :],
                            rhs=w_sb[:, kk, o0 : o0 + cw],
                            start=(kk == 0),
                            stop=(kk == KT - 1),
                        )
                    ysb = ypool.tile([128, 512], f32, tag="ysb")
                    nc.vector.scalar_tensor_tensor(
                        out=ysb[:, :cw],
                        in0=ps[:, :cw],
                        scalar=xs_all[:, mt : mt + 1],
                        in1=sc_bc[:, o0 : o0 + cw],
                        op0=Alu.mult,
                        op1=Alu.mult,
                    )
                    nc.gpsimd.tensor_tensor(
                        out=ysb[:, :cw],
                        in0=ysb[:, :cw],
                        in1=bias_bc[:, o0 : o0 + cw],
                        op=Alu.add,
                    )
                    nc.scalar.dma_start(
                        out=y_d[ms, o0 : o0 + cw], in_=ysb[:, :cw]
                    )

            # software pipeline: phase1 runs 2 (rep, mt)-steps ahead of phase2
            AHEAD = 2
            steps = [(rep, mt) for rep in range(nrep) for mt in range(MT)]
            for i in range(len(steps) + AHEAD):
                if i < len(steps):
                    phase1(*steps[i])
                if i >= AHEAD:
                    phase2(*steps[i - AHEAD])

            if debug_dump:
                    nc.sync.dma_start(out=dbg["dbg_scales"][:, 0:MT], in_=am_all)
                    nc.sync.dma_start(
                        out=dbg["dbg_scales"][:, MT : 2 * MT], in_=xs_all
                    )
                    nc.sync.dma_start(
                        out=dbg["dbg_scales"][:, 2 * MT : 3 * MT], in_=recip_all
                    )
                    nc.sync.dma_start(out=dbg["dbg_q0"], in_=q_tiles[0][:, :, :])
                    nc.sync.dma_start(out=dbg["dbg_q3"], in_=q_tiles[3][:, :, :])
                    nc.sync.dma_start(out=dbg["dbg_w0"], in_=w_sb[:, 0, :])
                    nc.sync.dma_start(out=dbg["dbg_w31"], in_=w_sb[:, 31, :])

    nc.compile()
    return nc


_NC_CACHE = None


def get_program():
    global _NC_CACHE
    if _NC_CACHE is None:
        _NC_CACHE = build_program()
    return _NC_CACHE


def make_in_maps(x, q_weight, scale_col, weight_cache, ind, bias):
    import ml_dtypes

    x2 = np.ascontiguousarray(np.asarray(x, dtype=np.float32).reshape(M, K))
    q_weight = np.asarray(q_weight, dtype=np.int32)
    scale_col = np.asarray(scale_col, dtype=np.float32).reshape(OUT)
    weight_cache = np.asarray(weight_cache, dtype=np.float32)
    ind_np = np.asarray(ind, dtype=np.int32).reshape(FP)
    bias_np = np.asarray(bias, dtype=np.float32).reshape(OUT)

    mask = np.ones(K, dtype=np.float32)
    mask[ind_np] = 0.0
    mask_bf = mask.astype(ml_dtypes.bfloat16).reshape(1, K)

    in_maps = []
    for c in range(N_CORES):
        sl = slice(c * OSH, (c + 1) * OSH)
        sc_sh = scale_col[sl]
        w_sh = q_weight[sl]
        cache_sh = weight_cache[sl]
        # Folded transposed weights: wT[k, o] = W[o, k], with ind-rows
        # replaced by sum-over-duplicates of cache[:, j]/sc (those W entries
        # only ever multiply the zeroed activation columns in the reference).
        wT = w_sh.T.astype(np.float32)  # (K, OSH)
        acc = np.zeros((K, OSH), dtype=np.float32)
        np.add.at(acc, ind_np, (cache_sh / sc_sh[:, None]).T)
        wT[mask == 0.0] = acc[mask == 0.0]
        w_pack = np.ascontiguousarray(
            wT.reshape(KT, 128, OSH).transpose(1, 0, 2).reshape(128, KT * OSH)
        ).astype(np.float16)
        in_maps.append(
            {
                "x_in": x2,
                "w_in": w_pack,
                "mask_in": mask_bf,
                "sc_in": np.ascontiguousarray(sc_sh.reshape(1, OSH)),
                "bias_in": np.ascontiguousarray(bias_np[sl].reshape(1, OSH)),
            }
        )
    return in_maps


def kernel(x, q_weight, scale_col, weight_cache, ind, bias):
    from concourse.bass_utils import run_bass_kernel_spmd

    nc = get_program()
    in_maps = make_in_maps(x, q_weight, scale_col, weight_cache, ind, bias)
    res = run_bass_kernel_spmd(nc, in_maps, core_ids=list(range(N_CORES)))
    shards = [res.results[c]["y_out"] for c in range(N_CORES)]
    y = np.concatenate(shards, axis=1)
    return y.reshape(8, 64, OUT).astype(np.float32)
